# revision 1
# baseline (speedup 1.0000x reference)
"""Trainium2 Bass kernel for nn_Decoder_38293928411158.

6-block cross-attention decoder: B=64, L=64, S=512, D=768, H=12, E=32, MLP 4x.
Data-parallel over batch across 8 NeuronCores (8 batch elements per core).
Feature-major on-device layout ([D, tokens]); bf16 GEMMs.

The K projection runs as fp8(e4m3) DoubleRow matmuls (2 rows/cycle) off an
SBUF-cached fp8 copy of the encoder states; quantizing Q/K is benign because
the softmax logits are tiny.  All scale corrections fold host-side: the
residual stream runs at 32x its true scale (LayerNorm is scale-invariant;
the rsqrt's scale arg maintains the factor), wk is pre-scaled by 32 into
the fp8 sweet spot, Q*K scale folds into the softmax exp scale, and the W1
scale folds into the gelu input scale.  Host divides the final output by 32.

Attention softmax processes head PAIRS on full 128-partition tiles (one exp
per pair); the normalize+transpose chain alternates between the two HWDGE
queues (SP and Activation) so transposes pipeline.
Self-contained: hardcodes all shapes/layouts.
"""
import os
import sys

sys.path.insert(0, "/opt/trn_rl_repo")

import ml_dtypes
import numpy as np

import concourse.bass as bass
import concourse.mybir as mybir
import concourse.tile as tile
from concourse.bass_utils import run_bass_kernel_spmd

f32 = mybir.dt.float32
f32r = mybir.dt.float32r
bf16 = mybir.dt.bfloat16
fp8 = mybir.dt.float8e4
AF = mybir.ActivationFunctionType
OP = mybir.AluOpType
DR = mybir.MatmulPerfMode.DoubleRow

NB, H, D, E, L, S, B = 6, 12, 768, 32, 64, 512, 64
EXP = 4
EPS = 1e-6
NCORES = 8
BC = B // NCORES          # batches per core
T = BC * L                # query tokens per core (512)
KC = D // 128             # 6 d-chunks
MC = (H * E) // 128       # 3 qkv-output chunks
HID = EXP * D             # 3072
HC = HID // 128           # 24
SC = S // 128             # 4 s-chunks
SCALE = float(D) ** -0.5

SW = 32.0                 # residual-stream device scale (lambda)
ISW = 1.0 / SW            # gelu input scale
ISW2 = 1.0 / (SW * SW)    # LN sqrt scale
EXP_SCALE = SCALE * ISW2  # Q and K both carry a factor of SW
FP8MAX = 240.0


def _split_waits(nc, max_waits=1):
    """Walrus codegen rejects instructions with >1 sem-wait; hoist extras
    into single-wait NoOps on the same engine, inserted just before."""
    n = 0
    cnt = 0
    for fn in nc.m.functions:
        for bb in fn.blocks:
            new_insts = []
            for inst in bb.instructions:
                si = inst.sync_info
                waits = list(si.on_wait) if (si is not None and si.on_wait) else []
                if len(waits) > max_waits:
                    head, tail = waits[:-max_waits], waits[-max_waits:]
                    for w in head:
                        cnt += 1
                        nop = mybir.InstNoOp(name=f"I-wsplit-{cnt}", ins=[], outs=[])
                        nop.engine = inst.engine
                        nop.bass_nofuse = True
                        nop.sync_info = mybir.SyncInfo(on_wait=[w], on_update=[])
                        new_insts.append(nop)
                        nc.register_instruction(nop, overwrite=True)
                    inst.sync_info = mybir.SyncInfo(
                        on_wait=tail, on_update=list(si.on_update or [])
                    )
                    n += 1
                new_insts.append(inst)
            bb.instructions[:] = new_insts
    return n


def build(nb_run=NB, bc=BC, ln_affine=True):
    t = bc * L
    nc = bass.Bass()
    enc_d = nc.dram_tensor("enc_d", (bc, 128, KC, S), bf16, kind="ExternalInput")
    enc8_d = nc.dram_tensor("enc8_d", (bc, 128, KC, S), fp8, kind="ExternalInput")
    x0_d = nc.dram_tensor("x0_d", (128, KC, t), f32r, kind="ExternalInput")
    wq_d = nc.dram_tensor("wq_d", (NB, 128, KC, H * E), bf16, kind="ExternalInput")
    wk_d = nc.dram_tensor("wk_d", (NB, 128, KC, H * E), fp8, kind="ExternalInput")
    wv_d = nc.dram_tensor("wv_d", (NB, 128, KC, H * E), bf16, kind="ExternalInput")
    wo_d = nc.dram_tensor("wo_d", (NB, 128, MC, D), bf16, kind="ExternalInput")
    w1_d = nc.dram_tensor("w1_d", (NB, HC, 128, KC, 128), bf16, kind="ExternalInput")
    w2_d = nc.dram_tensor("w2_d", (NB, KC, 128, HC, 128), bf16, kind="ExternalInput")
    l1g_d = nc.dram_tensor("l1g_d", (128, KC, NB), f32, kind="ExternalInput")
    l1b_d = nc.dram_tensor("l1b_d", (128, KC, NB), f32, kind="ExternalInput")
    l2g_d = nc.dram_tensor("l2g_d", (128, KC, NB), f32, kind="ExternalInput")
    l2b_d = nc.dram_tensor("l2b_d", (128, KC, NB), f32, kind="ExternalInput")
    out_d = nc.dram_tensor("out_d", (128, KC, t), f32r, kind="ExternalOutput")

    with tile.TileContext(nc) as tc:
        with (
            tc.tile_pool(name="consts", bufs=1) as consts,
            tc.tile_pool(name="acts", bufs=2) as actp,
            tc.tile_pool(name="wp", bufs=2) as wpool,
            tc.tile_pool(name="wstream", bufs=3) as wsp,
            tc.tile_pool(name="ep", bufs=3) as epool,
            tc.tile_pool(name="bp", bufs=2) as bpool,
            tc.tile_pool(name="sm", bufs=4) as smp,
            tc.tile_pool(name="pbig", bufs=3, space="PSUM") as pbig,
            tc.tile_pool(name="ppa", bufs=3, space="PSUM") as ppa,
            tc.tile_pool(name="ppo", bufs=2, space="PSUM") as ppo,
        ):
            # ---- constants ----
            ones1_f = consts.tile([128, 1], f32)
            nc.vector.memset(ones1_f[:], 1.0)
            ones1 = consts.tile([128, 1], f32r)
            nc.vector.tensor_copy(ones1[:], ones1_f[:])
            onesr_f = consts.tile([1, 128], f32)
            nc.vector.memset(onesr_f[:], 1.0)
            onesr = consts.tile([1, 128], f32r)
            nc.vector.tensor_copy(onesr[:], onesr_f[:])
            eps_t = consts.tile([1, 1], f32)
            nc.vector.memset(eps_t[:], EPS)
            l1g = consts.tile([128, KC, NB], f32)
            nc.sync.dma_start(l1g[:], l1g_d[:])
            l1b = consts.tile([128, KC, NB], f32)
            nc.sync.dma_start(l1b[:], l1b_d[:])
            l2g = consts.tile([128, KC, NB], f32)
            nc.sync.dma_start(l2g[:], l2g_d[:])
            l2b = consts.tile([128, KC, NB], f32)
            nc.sync.dma_start(l2b[:], l2b_d[:])

            # ---- fp8 encoder states cached in SBUF across all blocks ----
            enc8_sb = consts.tile([128, bc, KC, S], fp8)
            for b in range(bc):
                nc.sync.dma_start(enc8_sb[:, b], enc8_d[b])

            # ---- initial x (residual f32r at SW scale) + bf16 matmul copy ----
            x = actp.tile([128, KC, t], f32r, tag="x", bufs=2)
            nc.sync.dma_start(x[:], x0_d[:])
            xb = actp.tile([128, KC, t], bf16, tag="xb", bufs=2)
            nc.scalar.copy(xb[:], x[:])

            def layernorm(r, g_t, b_t, blk):
                """r: [128, KC, t] f32r residual-summed input at SW scale.
                Returns (xn f32r at SW scale, xb bf16 copy)."""
                ps_sum = pbig.tile([128, 512], f32, tag="big")
                ps_sq = pbig.tile([128, 512], f32, tag="big")
                for k in range(KC):
                    r2 = actp.tile([128, t], f32r, tag="r2", bufs=2)
                    nc.vector.tensor_mul(r2[:], r[:, k, :], r[:, k, :])
                    nc.tensor.matmul(
                        ps_sum[0:1, :t], ones1[:], r[:, k, :],
                        start=(k == 0), stop=(k == KC - 1),
                    )
                    nc.tensor.matmul(
                        ps_sq[0:1, :t], ones1[:], r2[:],
                        start=(k == 0), stop=(k == KC - 1),
                    )
                nm = smp.tile([1, t], f32, tag="nm", bufs=1)
                nc.vector.tensor_scalar_mul(nm[:], ps_sum[0:1, :t], -1.0 / D)
                var = smp.tile([1, t], f32, tag="var", bufs=1)
                nc.vector.tensor_mul(var[:], nm[:], nm[:])
                std = smp.tile([1, t], f32, tag="std", bufs=1)
                nc.vector.tensor_scalar_mul(std[:], ps_sq[0:1, :t], 1.0 / D)
                nc.vector.tensor_tensor(var[:], std[:], var[:], OP.subtract)
                # var is at SW^2 scale; sqrt(var*ISW2 + eps) is the true std,
                # so (r - m) * rstd below lands back at SW scale.
                nc.scalar.activation(std[:], var[:], AF.Sqrt, bias=eps_t[:],
                                     scale=ISW2)
                nc.vector.reciprocal(var[:], std[:])
                nm_r = smp.tile([1, t], f32r, tag="nm_r", bufs=1)
                nc.vector.tensor_copy(nm_r[:], nm[:])
                rstd_r = smp.tile([1, t], f32r, tag="rstd_r", bufs=1)
                nc.vector.tensor_copy(rstd_r[:], var[:])
                pnm = pbig.tile([128, 512], f32, tag="big")
                nc.tensor.matmul(pnm[:, :t], onesr[:], nm_r[:], start=True, stop=True)
                prs = pbig.tile([128, 512], f32, tag="big")
                nc.tensor.matmul(prs[:, :t], onesr[:], rstd_r[:], start=True, stop=True)
                xn = actp.tile([128, KC, t], f32r, tag="x", bufs=2)
                xnb = actp.tile([128, KC, t], bf16, tag="xb", bufs=2)
                for k in range(KC):
                    nc.vector.tensor_add(xn[:, k, :], r[:, k, :], pnm[:, :t])
                    nc.vector.tensor_mul(xn[:, k, :], xn[:, k, :], prs[:, :t])
                    if ln_affine:
                        nc.vector.tensor_scalar(
                            xn[:, k, :], xn[:, k, :],
                            g_t[:, k, blk : blk + 1], b_t[:, k, blk : blk + 1],
                            OP.mult, OP.add,
                        )
                    # per-chunk cast so downstream matmuls chase chunk-by-chunk
                    nc.scalar.copy(xnb[:, k, :], xn[:, k, :])
                return xn, xnb

            for blk in range(nb_run):
                # ---- per-block weights ----
                wq_t = wpool.tile([128, KC, H * E], bf16, tag="wq", bufs=1)
                nc.sync.dma_start(wq_t[:], wq_d[blk])
                wk_t = wpool.tile([128, KC, H * E], fp8, tag="wk", bufs=1)
                nc.sync.dma_start(wk_t[:], wk_d[blk])
                wv_t = wpool.tile([128, KC, H * E], bf16, tag="wv", bufs=1)
                nc.sync.dma_start(wv_t[:], wv_d[blk])
                wo_t = wpool.tile([128, MC, D], bf16, tag="wo", bufs=1)
                nc.sync.dma_start(wo_t[:], wo_d[blk])

                def make_k(b):
                    """fp8 DoubleRow off the cached fp8 encoder."""
                    kt = epool.tile([128, MC, S], bf16, tag="kt", bufs=2)
                    for m in range(MC):
                        pk = pbig.tile([128, 512], f32, tag="big")
                        for kp in range(KC // 2):
                            nc.tensor.matmul(
                                pk[:],
                                wk_t[:, 2 * kp : 2 * kp + 2, 128 * m : 128 * (m + 1)],
                                enc8_sb[:, b, 2 * kp : 2 * kp + 2, :],
                                start=(kp == 0), stop=(kp == KC // 2 - 1),
                                perf_mode=DR,
                            )
                        nc.vector.tensor_copy(kt[:, m, :], pk[:])
                    return kt

                def make_v(b):
                    enc_t = epool.tile([128, KC, S], bf16, tag="enc", bufs=2)
                    nc.sync.dma_start(enc_t[:], enc_d[b])
                    v_t = epool.tile([128, SC, H, E], bf16, tag="v", bufs=3)
                    for sc in range(SC):
                        pv = pbig.tile([128, 512], f32, tag="big")
                        for k in range(KC):
                            nc.tensor.matmul(
                                pv[:, : H * E],
                                enc_t[:, k, 128 * sc : 128 * (sc + 1)],
                                wv_t[:, k, :], start=(k == 0), stop=(k == KC - 1),
                            )
                        nc.vector.tensor_copy(
                            v_t[:, sc, :, :],
                            pv[:, : H * E].rearrange("p (h e) -> p h e", e=E),
                        )
                    return v_t

                ot = bpool.tile([128, MC, t], bf16, tag="ot", bufs=1)
                prev = None  # (b, v_t, at2) pending AV work

                def issue_av(pend):
                    b, v_t, at2 = pend
                    for g in range(MC):
                        po = ppo.tile([128, L], f32, tag="po")
                        for jj in range(4):
                            h = 4 * g + jj
                            for sc in range(SC):
                                nc.tensor.matmul(
                                    po[32 * jj : 32 * (jj + 1), :],
                                    v_t[:, sc, h, :],
                                    at2[:, h // 2, sc, 64 * (h % 2) : 64 * (h % 2) + 64],
                                    start=(sc == 0), stop=(sc == SC - 1),
                                    tile_position=(0, 32 * jj),
                                )
                        nc.vector.tensor_copy(ot[:, g, L * b : L * (b + 1)], po[:])

                # ---- Q^T for all local batches (needs x from prev LN) ----
                qt = bpool.tile([128, MC, t], bf16, tag="qt", bufs=1)
                for m in range(MC):
                    pq = pbig.tile([128, 512], f32, tag="big")
                    for k in range(KC):
                        nc.tensor.matmul(
                            pq[:, :t], wq_t[:, k, 128 * m : 128 * (m + 1)],
                            xb[:, k, :], start=(k == 0), stop=(k == KC - 1),
                        )
                    nc.vector.tensor_copy(qt[:, m, :], pq[:, :t])

                for b in range(bc):
                    kt = make_k(b)
                    v_t = make_v(b)
                    # logits + softmax + transpose per head PAIR; the
                    # normalize+transpose chain is fully interleaved so the
                    # two DMA queues (SP + Activation) start early.
                    at2 = epool.tile([128, H // 2, SC, 128], bf16, tag="at", bufs=3)
                    for j in range(H // 2):
                        pa2 = ppa.tile([128, S], f32, tag="pa")
                        for hl in range(2):
                            h = 2 * j + hl
                            rp = 32 * (h % 4)
                            nc.tensor.matmul(
                                pa2[64 * hl : 64 * hl + 64, :],
                                qt[rp : rp + 32, h // 4, L * b : L * (b + 1)],
                                kt[rp : rp + 32, h // 4, :], start=True, stop=True,
                                tile_position=(rp, 64 * hl),
                            )
                        aexp = smp.tile([128, S], bf16, tag="aexp", bufs=3)
                        sums = smp.tile([128, 1], f32, tag="sums", bufs=3)
                        nc.scalar.activation(
                            aexp[:], pa2[:], AF.Exp, scale=EXP_SCALE,
                            accum_out=sums[:],
                        )
                        rec = smp.tile([128, 1], f32, tag="rec", bufs=3)
                        nc.vector.reciprocal(rec[:], sums[:])
                        anorm = smp.tile([128, S], bf16, tag="anorm", bufs=3)
                        nc.vector.tensor_scalar_mul(anorm[:], aexp[:], rec[:])
                        eng = nc.sync if j % 2 == 0 else nc.scalar
                        eng.dma_start_transpose(at2[:, j], anorm[:])
                    if prev is not None:
                        issue_av(prev)
                    prev = (b, v_t, at2)
                issue_av(prev)

                # ---- output projection + residual + LN1 ----
                r1 = actp.tile([128, KC, t], f32r, tag="r", bufs=1)
                for m in range(KC):
                    px = pbig.tile([128, 512], f32, tag="big")
                    for kc2 in range(MC):
                        nc.tensor.matmul(
                            px[:, :t], wo_t[:, kc2, 128 * m : 128 * (m + 1)],
                            ot[:, kc2, :], start=(kc2 == 0), stop=(kc2 == MC - 1),
                        )
                    nc.vector.tensor_add(r1[:, m, :], x[:, m, :], px[:, :t])
                xn, xnb = layernorm(r1, l1g, l1b, blk)
                # ---- MLP ----
                h_all = actp.tile([128, HC, t], bf16, tag="h", bufs=1)
                for m in range(HC):
                    w1s = wsp.tile([128, KC, 128], bf16, tag="w1", bufs=3)
                    nc.sync.dma_start(w1s[:], w1_d[blk, m])
                    ph = pbig.tile([128, 512], f32, tag="big")
                    for k in range(KC):
                        nc.tensor.matmul(
                            ph[:, :t], w1s[:, k, :], xnb[:, k, :],
                            start=(k == 0), stop=(k == KC - 1),
                        )
                    nc.scalar.activation(h_all[:, m, :], ph[:, :t], AF.Gelu,
                                         scale=ISW)
                r2t = actp.tile([128, KC, t], f32r, tag="r", bufs=1)
                for j in range(KC):
                    w2s = wsp.tile([128, HC, 128], bf16, tag="w2", bufs=2)
                    nc.sync.dma_start(w2s[:], w2_d[blk, j])
                    pxx = pbig.tile([128, 512], f32, tag="big")
                    for m in range(HC):
                        nc.tensor.matmul(
                            pxx[:, :t], w2s[:, m, :], h_all[:, m, :],
                            start=(m == 0), stop=(m == HC - 1),
                        )
                    nc.vector.tensor_add(r2t[:, j, :], xn[:, j, :], pxx[:, :t])
                x, xb = layernorm(r2t, l2g, l2b, blk)

            nc.sync.dma_start(out_d[:], x[:])

    _split_waits(nc)
    return nc


def _prep_inputs(inputs):
    """Host-side: full inputs -> per-core device-layout arrays.

    The residual stream runs at 32x on device: x0, wk, wo, w2 and the LN
    biases are pre-scaled by 32 (wk additionally quantized to fp8 e4m3)."""
    f = np.float32
    bf = ml_dtypes.bfloat16
    q8 = ml_dtypes.float8_e4m3
    enc = np.asarray(inputs["encoder_outputs"], f)           # [64, 512, 768]
    pos = np.asarray(inputs["pos_emb"], f)                   # [1, 64, 768]
    wq = np.asarray(inputs["wq"], f)                         # [6, 12, 768, 32]
    wk = np.asarray(inputs["wk"], f)
    wv = np.asarray(inputs["wv"], f)
    wo = np.asarray(inputs["wo"], f)                         # [6, 384, 768]
    w1 = np.asarray(inputs["w1"], f)                         # [6, 768, 3072]
    w2 = np.asarray(inputs["w2"], f)                         # [6, 3072, 768]

    def qkv_layout(w, dt, scale=1.0):
        # [6, 12, 768, 32] -> flat [6, 768, 384] -> [6, 128, 6, 384]
        wf = w.transpose(0, 2, 1, 3).reshape(NB, D, H * E) * scale
        return np.clip(np.ascontiguousarray(
            wf.reshape(NB, KC, 128, H * E).transpose(0, 2, 1, 3)
        ), -FP8MAX, FP8MAX).astype(dt)

    def ln_layout(p, scale=1.0):
        return np.ascontiguousarray(
            (np.asarray(p, f) * scale).reshape(NB, KC, 128).transpose(2, 1, 0)
        )

    encT = enc.transpose(0, 2, 1)                            # [64, 768, 512]
    enc_l = np.ascontiguousarray(
        encT.reshape(B, KC, 128, S).transpose(0, 2, 1, 3)
    )                                                        # [64, 128, 6, 512]
    x0T = np.tile(pos[0].T, (1, BC)) * SW                    # [768, 512] at SW
    x0_l = np.ascontiguousarray(x0T.reshape(KC, 128, T).transpose(1, 0, 2))

    common = {
        "x0_d": x0_l.astype(f),
        "wq_d": qkv_layout(wq, bf),
        "wk_d": qkv_layout(wk, q8, SW),
        "wv_d": qkv_layout(wv, bf),
        "wo_d": (SW * np.ascontiguousarray(
            wo.reshape(NB, MC, 128, D).transpose(0, 2, 1, 3)
        )).astype(bf),
        "w1_d": np.ascontiguousarray(
            w1.reshape(NB, KC, 128, HC, 128).transpose(0, 3, 2, 1, 4)
        ).astype(bf),
        "w2_d": (SW * np.ascontiguousarray(
            w2.reshape(NB, HC, 128, KC, 128).transpose(0, 3, 2, 1, 4)
        )).astype(bf),
        "l1g_d": ln_layout(inputs["ln1_g"]),
        "l1b_d": ln_layout(inputs["ln1_b"], SW),
        "l2g_d": ln_layout(inputs["ln2_g"]),
        "l2b_d": ln_layout(inputs["ln2_b"], SW),
    }
    in_maps = []
    for c in range(NCORES):
        m = dict(common)
        sl = enc_l[c * BC : (c + 1) * BC]
        m["enc_d"] = sl.astype(bf)
        m["enc8_d"] = np.clip(sl, -FP8MAX, FP8MAX).astype(q8)
        in_maps.append(m)
    return in_maps


_CACHED = {}


def kernel(**inputs) -> np.ndarray:
    ln_affine = not (
        np.all(np.asarray(inputs["ln1_g"]) == 1.0)
        and np.all(np.asarray(inputs["ln2_g"]) == 1.0)
        and np.all(np.asarray(inputs["ln1_b"]) == 0.0)
        and np.all(np.asarray(inputs["ln2_b"]) == 0.0)
    )
    key = ("nc", ln_affine)
    if key not in _CACHED:
        _CACHED[key] = build(ln_affine=ln_affine)
    nc = _CACHED[key]
    in_maps = _prep_inputs(inputs)
    res = run_bass_kernel_spmd(
        nc, in_maps, core_ids=list(range(NCORES)),
        trace=bool(int(os.environ.get("KERNEL_TRACE", "0"))),
    )
    _CACHED["last_result"] = res
    outs = []
    for c in range(NCORES):
        o = res.results[c]["out_d"]                          # [128, 6, 512] f32
        xT = o.transpose(1, 0, 2).reshape(D, T)              # [768, 512]
        outs.append(xT.T.reshape(BC, L, D))                  # [8, 64, 768]
    return (np.concatenate(outs, 0) * np.float32(1.0 / SW)).astype(np.float32)



# revision 13
# speedup vs baseline: 1.2056x; 1.2056x over previous
"""Trainium2 Bass kernel for nn_Decoder_38293928411158.

6-block cross-attention decoder: B=64, L=64, S=512, D=768, H=12, E=32, MLP 4x.
Data-parallel over batch across 8 NeuronCores (8 batch elements per core).
Feature-major on-device layout ([D, tokens]); bf16 GEMMs.

The K projection runs as fp8(e4m3) DoubleRow matmuls (2 rows/cycle) off an
SBUF-cached fp8 copy of the encoder states; quantizing Q/K is benign because
the softmax logits are tiny.  All scale corrections fold host-side: the
residual stream runs at 32x its true scale (LayerNorm is scale-invariant;
the rsqrt's scale arg maintains the factor), wk is pre-scaled by 32 into
the fp8 sweet spot, Q*K scale folds into the softmax exp scale, and the W1
scale folds into the gelu input scale.  Host divides the final output by 32.

Attention softmax processes head PAIRS on full 128-partition tiles (one exp
per pair); the normalize+transpose chain alternates between the two HWDGE
queues (SP and Activation) so transposes pipeline.
Self-contained: hardcodes all shapes/layouts.
"""
import os
import sys

sys.path.insert(0, "/opt/trn_rl_repo")

import ml_dtypes
import numpy as np

import concourse.bass as bass
import concourse.mybir as mybir
import concourse.tile as tile
from concourse.bass_utils import run_bass_kernel_spmd

f32 = mybir.dt.float32
f32r = mybir.dt.float32r
bf16 = mybir.dt.bfloat16
fp8 = mybir.dt.float8e4
AF = mybir.ActivationFunctionType
OP = mybir.AluOpType
DR = mybir.MatmulPerfMode.DoubleRow

NB, H, D, E, L, S, B = 6, 12, 768, 32, 64, 512, 64
EXP = 4
EPS = 1e-6
NCORES = 8
BC = B // NCORES          # batches per core
T = BC * L                # query tokens per core (512)
KC = D // 128             # 6 d-chunks
MC = (H * E) // 128       # 3 qkv-output chunks
HID = EXP * D             # 3072
HC = HID // 128           # 24
SC = S // 128             # 4 s-chunks
SCALE = float(D) ** -0.5

SW = 32.0                 # residual-stream device scale (lambda)
ISW = 1.0 / SW            # gelu input scale
ISW2 = 1.0 / (SW * SW)    # LN sqrt scale
EXP_SCALE = SCALE * ISW2  # Q and K both carry a factor of SW
FP8MAX = 240.0


def _split_waits(nc, max_waits=1):
    """Walrus codegen rejects instructions with >1 sem-wait; hoist extras
    into single-wait NoOps on the same engine, inserted just before."""
    n = 0
    cnt = 0
    for fn in nc.m.functions:
        for bb in fn.blocks:
            new_insts = []
            for inst in bb.instructions:
                si = inst.sync_info
                waits = list(si.on_wait) if (si is not None and si.on_wait) else []
                if len(waits) > max_waits:
                    head, tail = waits[:-max_waits], waits[-max_waits:]
                    for w in head:
                        cnt += 1
                        nop = mybir.InstNoOp(name=f"I-wsplit-{cnt}", ins=[], outs=[])
                        nop.engine = inst.engine
                        nop.bass_nofuse = True
                        nop.sync_info = mybir.SyncInfo(on_wait=[w], on_update=[])
                        new_insts.append(nop)
                        nc.register_instruction(nop, overwrite=True)
                    inst.sync_info = mybir.SyncInfo(
                        on_wait=tail, on_update=list(si.on_update or [])
                    )
                    n += 1
                new_insts.append(inst)
            bb.instructions[:] = new_insts
    return n


def build(nb_run=NB, bc=BC, ln_affine=True):
    t = bc * L
    nc = bass.Bass()
    enc8_d = nc.dram_tensor("enc8_d", (bc, 128, KC, S), fp8, kind="ExternalInput")
    x0_d = nc.dram_tensor("x0_d", (128, KC, t), f32r, kind="ExternalInput")
    wq_d = nc.dram_tensor("wq_d", (NB, 128, KC, H * E), bf16, kind="ExternalInput")
    wk_d = nc.dram_tensor("wk_d", (NB, 128, KC, H * E), fp8, kind="ExternalInput")
    wv_d = nc.dram_tensor("wv_d", (NB, 128, KC, H * E), fp8, kind="ExternalInput")
    wo_d = nc.dram_tensor("wo_d", (NB, 128, MC, D), bf16, kind="ExternalInput")
    w1_d = nc.dram_tensor("w1_d", (NB, HC, 128, KC, 128), bf16, kind="ExternalInput")
    w2_d = nc.dram_tensor("w2_d", (NB, KC, 128, HC, 128), bf16, kind="ExternalInput")
    l1g_d = nc.dram_tensor("l1g_d", (128, KC, NB), f32, kind="ExternalInput")
    l1b_d = nc.dram_tensor("l1b_d", (128, KC, NB), f32, kind="ExternalInput")
    l2g_d = nc.dram_tensor("l2g_d", (128, KC, NB), f32, kind="ExternalInput")
    l2b_d = nc.dram_tensor("l2b_d", (128, KC, NB), f32, kind="ExternalInput")
    out_d = nc.dram_tensor("out_d", (128, KC, t), f32r, kind="ExternalOutput")

    with tile.TileContext(nc) as tc:
        with (
            tc.tile_pool(name="consts", bufs=1) as consts,
            tc.tile_pool(name="acts", bufs=2) as actp,
            tc.tile_pool(name="wp", bufs=2) as wpool,
            tc.tile_pool(name="wstream", bufs=3) as wsp,
            tc.tile_pool(name="ep", bufs=3) as epool,
            tc.tile_pool(name="bp", bufs=2) as bpool,
            tc.tile_pool(name="sm", bufs=4) as smp,
            tc.tile_pool(name="pbig", bufs=3, space="PSUM") as pbig,
            tc.tile_pool(name="ppa", bufs=3, space="PSUM") as ppa,
            tc.tile_pool(name="ppo", bufs=2, space="PSUM") as ppo,
        ):
            # ---- constants ----
            ones1_f = consts.tile([128, 1], f32)
            nc.vector.memset(ones1_f[:], 1.0)
            ones1 = consts.tile([128, 1], f32r)
            nc.vector.tensor_copy(ones1[:], ones1_f[:])
            onesr_f = consts.tile([1, 128], f32)
            nc.vector.memset(onesr_f[:], 1.0)
            onesr = consts.tile([1, 128], f32r)
            nc.vector.tensor_copy(onesr[:], onesr_f[:])
            eps_t = consts.tile([1, 1], f32)
            nc.vector.memset(eps_t[:], EPS)
            l1g = consts.tile([128, KC, NB], f32)
            nc.sync.dma_start(l1g[:], l1g_d[:])
            l1b = consts.tile([128, KC, NB], f32)
            nc.sync.dma_start(l1b[:], l1b_d[:])
            l2g = consts.tile([128, KC, NB], f32)
            nc.sync.dma_start(l2g[:], l2g_d[:])
            l2b = consts.tile([128, KC, NB], f32)
            nc.sync.dma_start(l2b[:], l2b_d[:])

            # ---- fp8 encoder states cached in SBUF across all blocks ----
            enc8_sb = consts.tile([128, bc, KC, S], fp8)
            for b in range(bc):
                nc.sync.dma_start(enc8_sb[:, b], enc8_d[b])

            # ---- initial x (residual f32r at SW scale) + bf16 matmul copy ----
            x = actp.tile([128, KC, t], f32r, tag="x", bufs=2)
            nc.sync.dma_start(x[:], x0_d[:])
            xb = actp.tile([128, KC, t], bf16, tag="xb", bufs=2)
            nc.scalar.copy(xb[:], x[:])

            def layernorm(r, g_t, b_t, blk, cast_dt=bf16, cast_tag="xb"):
                """r: [128, KC, t] f32r residual-summed input at SW scale.
                Returns (xn f32r at SW scale, cast_dt copy for matmuls)."""
                ps_sum = pbig.tile([128, 512], f32, tag="big")
                ps_sq = pbig.tile([128, 512], f32, tag="big")
                for k in range(KC):
                    r2 = actp.tile([128, t], f32r, tag="r2", bufs=2)
                    nc.vector.tensor_mul(r2[:], r[:, k, :], r[:, k, :])
                    nc.tensor.matmul(
                        ps_sum[0:1, :t], ones1[:], r[:, k, :],
                        start=(k == 0), stop=(k == KC - 1),
                    )
                    nc.tensor.matmul(
                        ps_sq[0:1, :t], ones1[:], r2[:],
                        start=(k == 0), stop=(k == KC - 1),
                    )
                nm = smp.tile([1, t], f32, tag="nm", bufs=1)
                nc.vector.tensor_scalar_mul(nm[:], ps_sum[0:1, :t], -1.0 / D)
                var = smp.tile([1, t], f32, tag="var", bufs=1)
                nc.vector.tensor_mul(var[:], nm[:], nm[:])
                std = smp.tile([1, t], f32, tag="std", bufs=1)
                nc.vector.tensor_scalar_mul(std[:], ps_sq[0:1, :t], 1.0 / D)
                nc.vector.tensor_tensor(var[:], std[:], var[:], OP.subtract)
                # var is at SW^2 scale; sqrt(var*ISW2 + eps) is the true std,
                # so (r - m) * rstd below lands back at SW scale.
                nc.scalar.activation(std[:], var[:], AF.Sqrt, bias=eps_t[:],
                                     scale=ISW2)
                nc.vector.reciprocal(var[:], std[:])
                nm_r = smp.tile([1, t], f32r, tag="nm_r", bufs=1)
                nc.vector.tensor_copy(nm_r[:], nm[:])
                rstd_r = smp.tile([1, t], f32r, tag="rstd_r", bufs=1)
                nc.vector.tensor_copy(rstd_r[:], var[:])
                pnm = pbig.tile([128, 512], f32, tag="big")
                nc.tensor.matmul(pnm[:, :t], onesr[:], nm_r[:], start=True, stop=True)
                prs = pbig.tile([128, 512], f32, tag="big")
                nc.tensor.matmul(prs[:, :t], onesr[:], rstd_r[:], start=True, stop=True)
                xn = actp.tile([128, KC, t], f32r, tag="x", bufs=2)
                xnb = actp.tile([128, KC, t], cast_dt, tag=cast_tag, bufs=2)
                for k in range(KC):
                    nc.vector.tensor_add(xn[:, k, :], r[:, k, :], pnm[:, :t])
                    nc.vector.tensor_mul(xn[:, k, :], xn[:, k, :], prs[:, :t])
                    if ln_affine:
                        nc.vector.tensor_scalar(
                            xn[:, k, :], xn[:, k, :],
                            g_t[:, k, blk : blk + 1], b_t[:, k, blk : blk + 1],
                            OP.mult, OP.add,
                        )
                    # per-chunk cast so downstream matmuls chase chunk-by-chunk
                    nc.scalar.copy(xnb[:, k, :], xn[:, k, :])
                return xn, xnb

            for blk in range(nb_run):
                # ---- per-block weights ----
                wq_t = wpool.tile([128, KC, H * E], bf16, tag="wq", bufs=1)
                nc.sync.dma_start(wq_t[:], wq_d[blk])
                wk_t = wpool.tile([128, KC, H * E], fp8, tag="wk", bufs=1)
                nc.sync.dma_start(wk_t[:], wk_d[blk])
                wv_t = wpool.tile([128, KC, H * E], fp8, tag="wv", bufs=1)
                nc.sync.dma_start(wv_t[:], wv_d[blk])
                wo_t = wpool.tile([128, MC, D], bf16, tag="wo", bufs=1)
                nc.sync.dma_start(wo_t[:], wo_d[blk])

                def make_k(b):
                    """fp8 DoubleRow off the cached fp8 encoder."""
                    kt = epool.tile([128, MC, S], bf16, tag="kt", bufs=2)
                    for m in range(MC):
                        pk = pbig.tile([128, 512], f32, tag="big")
                        for kp in range(KC // 2):
                            nc.tensor.matmul(
                                pk[:],
                                wk_t[:, 2 * kp : 2 * kp + 2, 128 * m : 128 * (m + 1)],
                                enc8_sb[:, b, 2 * kp : 2 * kp + 2, :],
                                start=(kp == 0), stop=(kp == KC // 2 - 1),
                                perf_mode=DR,
                            )
                        nc.vector.tensor_copy(kt[:, m, :], pk[:])
                    return kt

                def make_v(b):
                    """fp8 DoubleRow off the cached fp8 encoder (wv at 32x)."""
                    v_t = epool.tile([128, SC, H, E], bf16, tag="v", bufs=3)
                    for sc in range(SC):
                        pv = pbig.tile([128, 512], f32, tag="big")
                        for kp in range(KC // 2):
                            nc.tensor.matmul(
                                pv[:, : H * E],
                                enc8_sb[:, b, 2 * kp : 2 * kp + 2,
                                        128 * sc : 128 * (sc + 1)],
                                wv_t[:, 2 * kp : 2 * kp + 2, :],
                                start=(kp == 0), stop=(kp == KC // 2 - 1),
                                perf_mode=DR,
                            )
                        nc.vector.tensor_copy(
                            v_t[:, sc, :, :],
                            pv[:, : H * E].rearrange("p (h e) -> p h e", e=E),
                        )
                    return v_t

                ot = bpool.tile([128, MC, t], bf16, tag="ot", bufs=1)
                prev = None  # (b, v_t, at2) pending AV work

                def issue_av(pend):
                    b, v_t, at2 = pend
                    for g in range(MC):
                        po = ppo.tile([128, L], f32, tag="po")
                        for jj in range(4):
                            h = 4 * g + jj
                            for sc in range(SC):
                                nc.tensor.matmul(
                                    po[32 * jj : 32 * (jj + 1), :],
                                    v_t[:, sc, h, :],
                                    at2[:, h // 2, sc, 64 * (h % 2) : 64 * (h % 2) + 64],
                                    start=(sc == 0), stop=(sc == SC - 1),
                                    tile_position=(0, 32 * jj),
                                )
                        nc.vector.tensor_scalar_mul(
                            ot[:, g, L * b : L * (b + 1)], po[:], ISW)

                # ---- Q^T for all local batches (needs x from prev LN) ----
                qt = bpool.tile([128, MC, t], bf16, tag="qt", bufs=1)
                for m in range(MC):
                    pq = pbig.tile([128, 512], f32, tag="big")
                    for k in range(KC):
                        nc.tensor.matmul(
                            pq[:, :t], wq_t[:, k, 128 * m : 128 * (m + 1)],
                            xb[:, k, :], start=(k == 0), stop=(k == KC - 1),
                        )
                    nc.vector.tensor_copy(qt[:, m, :], pq[:, :t])

                for b in range(bc):
                    kt = make_k(b)
                    v_t = make_v(b)
                    # logits + softmax + transpose per head PAIR; the
                    # normalize+transpose chain is fully interleaved so the
                    # two DMA queues (SP + Activation) start early.
                    at2 = epool.tile([128, H // 2, SC, 128], bf16, tag="at", bufs=3)
                    for j in range(H // 2):
                        pa2 = ppa.tile([128, S], f32, tag="pa")
                        for hl in range(2):
                            h = 2 * j + hl
                            rp = 32 * (h % 4)
                            nc.tensor.matmul(
                                pa2[64 * hl : 64 * hl + 64, :],
                                qt[rp : rp + 32, h // 4, L * b : L * (b + 1)],
                                kt[rp : rp + 32, h // 4, :], start=True, stop=True,
                                tile_position=(rp, 64 * hl),
                            )
                        aexp = smp.tile([128, S], bf16, tag="aexp", bufs=3)
                        sums = smp.tile([128, 1], f32, tag="sums", bufs=3)
                        nc.scalar.activation(
                            aexp[:], pa2[:], AF.Exp, scale=EXP_SCALE,
                            accum_out=sums[:],
                        )
                        rec = smp.tile([128, 1], f32, tag="rec", bufs=3)
                        nc.vector.reciprocal(rec[:], sums[:])
                        anorm = smp.tile([128, S], bf16, tag="anorm", bufs=3)
                        nc.vector.tensor_scalar_mul(anorm[:], aexp[:], rec[:])
                        eng = nc.sync if j % 2 == 0 else nc.scalar
                        eng.dma_start_transpose(at2[:, j], anorm[:])
                    if prev is not None:
                        issue_av(prev)
                    prev = (b, v_t, at2)
                issue_av(prev)

                # ---- output projection + residual + LN1 ----
                r1 = actp.tile([128, KC, t], f32r, tag="r", bufs=1)
                for m in range(KC):
                    px = pbig.tile([128, 512], f32, tag="big")
                    for kc2 in range(MC):
                        nc.tensor.matmul(
                            px[:, :t], wo_t[:, kc2, 128 * m : 128 * (m + 1)],
                            ot[:, kc2, :], start=(kc2 == 0), stop=(kc2 == MC - 1),
                        )
                    nc.vector.tensor_add(r1[:, m, :], x[:, m, :], px[:, :t])
                xn, xnb = layernorm(r1, l1g, l1b, blk)
                # ---- MLP ----
                h_all = actp.tile([128, HC, t], bf16, tag="h", bufs=1)
                for m in range(HC):
                    w1s = wsp.tile([128, KC, 128], bf16, tag="w1", bufs=3)
                    nc.sync.dma_start(w1s[:], w1_d[blk, m])
                    ph = pbig.tile([128, 512], f32, tag="big")
                    for k in range(KC):
                        nc.tensor.matmul(
                            ph[:, :t], w1s[:, k, :], xnb[:, k, :],
                            start=(k == 0), stop=(k == KC - 1),
                        )
                    nc.scalar.activation(h_all[:, m, :], ph[:, :t], AF.Gelu,
                                         scale=ISW)
                r2t = actp.tile([128, KC, t], f32r, tag="r", bufs=1)
                for j in range(KC):
                    w2s = wsp.tile([128, HC, 128], bf16, tag="w2", bufs=2)
                    nc.sync.dma_start(w2s[:], w2_d[blk, j])
                    pxx = pbig.tile([128, 512], f32, tag="big")
                    for m in range(HC):
                        nc.tensor.matmul(
                            pxx[:, :t], w2s[:, m, :], h_all[:, m, :],
                            start=(m == 0), stop=(m == HC - 1),
                        )
                    nc.vector.tensor_add(r2t[:, j, :], xn[:, j, :], pxx[:, :t])
                x, xb = layernorm(r2t, l2g, l2b, blk)

            nc.sync.dma_start(out_d[:], x[:])

    _split_waits(nc)
    return nc


def _prep_inputs(inputs):
    """Host-side: full inputs -> per-core device-layout arrays.

    The residual stream runs at 32x on device: x0, wk, wo, w2 and the LN
    biases are pre-scaled by 32 (wk additionally quantized to fp8 e4m3)."""
    f = np.float32
    bf = ml_dtypes.bfloat16
    q8 = ml_dtypes.float8_e4m3
    enc = np.asarray(inputs["encoder_outputs"], f)           # [64, 512, 768]
    pos = np.asarray(inputs["pos_emb"], f)                   # [1, 64, 768]
    wq = np.asarray(inputs["wq"], f)                         # [6, 12, 768, 32]
    wk = np.asarray(inputs["wk"], f)
    wv = np.asarray(inputs["wv"], f)
    wo = np.asarray(inputs["wo"], f)                         # [6, 384, 768]
    w1 = np.asarray(inputs["w1"], f)                         # [6, 768, 3072]
    w2 = np.asarray(inputs["w2"], f)                         # [6, 3072, 768]

    def qkv_layout(w, dt, scale=1.0):
        # [6, 12, 768, 32] -> flat [6, 768, 384] -> [6, 128, 6, 384]
        wf = w.transpose(0, 2, 1, 3).reshape(NB, D, H * E) * scale
        return np.clip(np.ascontiguousarray(
            wf.reshape(NB, KC, 128, H * E).transpose(0, 2, 1, 3)
        ), -FP8MAX, FP8MAX).astype(dt)

    def ln_layout(p, scale=1.0):
        return np.ascontiguousarray(
            (np.asarray(p, f) * scale).reshape(NB, KC, 128).transpose(2, 1, 0)
        )

    encT = enc.transpose(0, 2, 1)                            # [64, 768, 512]
    enc_l = np.ascontiguousarray(
        encT.reshape(B, KC, 128, S).transpose(0, 2, 1, 3)
    )                                                        # [64, 128, 6, 512]
    x0T = np.tile(pos[0].T, (1, BC)) * SW                    # [768, 512] at SW
    x0_l = np.ascontiguousarray(x0T.reshape(KC, 128, T).transpose(1, 0, 2))

    common = {
        "x0_d": x0_l.astype(f),
        "wq_d": qkv_layout(wq, bf),
        "wk_d": qkv_layout(wk, q8, SW),
        "wv_d": qkv_layout(wv, q8, SW),
        "wo_d": (SW * np.ascontiguousarray(
            wo.reshape(NB, MC, 128, D).transpose(0, 2, 1, 3)
        )).astype(bf),
        "w1_d": np.ascontiguousarray(
            w1.reshape(NB, KC, 128, HC, 128).transpose(0, 3, 2, 1, 4)
        ).astype(bf),
        "w2_d": (SW * np.ascontiguousarray(
            w2.reshape(NB, HC, 128, KC, 128).transpose(0, 3, 2, 1, 4)
        )).astype(bf),
        "l1g_d": ln_layout(inputs["ln1_g"]),
        "l1b_d": ln_layout(inputs["ln1_b"], SW),
        "l2g_d": ln_layout(inputs["ln2_g"]),
        "l2b_d": ln_layout(inputs["ln2_b"], SW),
    }
    in_maps = []
    for c in range(NCORES):
        m = dict(common)
        sl = enc_l[c * BC : (c + 1) * BC]
        m["enc8_d"] = np.clip(sl, -FP8MAX, FP8MAX).astype(q8)
        in_maps.append(m)
    return in_maps


_CACHED = {}


def kernel(**inputs) -> np.ndarray:
    ln_affine = not (
        np.all(np.asarray(inputs["ln1_g"]) == 1.0)
        and np.all(np.asarray(inputs["ln2_g"]) == 1.0)
        and np.all(np.asarray(inputs["ln1_b"]) == 0.0)
        and np.all(np.asarray(inputs["ln2_b"]) == 0.0)
    )
    key = ("nc", ln_affine)
    if key not in _CACHED:
        _CACHED[key] = build(ln_affine=ln_affine)
    nc = _CACHED[key]
    in_maps = _prep_inputs(inputs)
    res = run_bass_kernel_spmd(
        nc, in_maps, core_ids=list(range(NCORES)),
        trace=bool(int(os.environ.get("KERNEL_TRACE", "0"))),
    )
    _CACHED["last_result"] = res
    outs = []
    for c in range(NCORES):
        o = res.results[c]["out_d"]                          # [128, 6, 512] f32
        xT = o.transpose(1, 0, 2).reshape(D, T)              # [768, 512]
        outs.append(xT.T.reshape(BC, L, D))                  # [8, 64, 768]
    return (np.concatenate(outs, 0) * np.float32(1.0 / SW)).astype(np.float32)

